# revision 51
# baseline (speedup 1.0000x reference)
"""Causal single-head attention forward (B=4, T=4096, C=256, H=64) on 8 NeuronCores.

Sharding: core = (batch, kv_parity).  Each core processes ALL queries of its
batch but only kv tiles (128 keys) whose global tile index has its parity
(even/odd interleave), which balances the causal workload across the two
cores of a batch.  Each core emits unnormalized numerator+denominator stacked
as ud[65, T] (rows 0:64 = (exp(S)@V)^T, row 64 = sum exp(S)); the host
merges: out = (u0+u1)/(d0+d1), transposed back.

The compiled program is parity-uniform; parity only enters through host-
prepared data (gathered xkv columns and the diagonal-pair mask values).

Engine plan per core:
- PE: bf16 projections (q from full x; k and v from the gathered parity
  columns, Wk|Wv packed in one stationary), bf16 S^T = K Q^T per 128-key
  tile.  AV: the diagonal pair runs in bf16 (it carries the peaked
  attention mass); every non-diagonal pair runs as ONE fp8e4 DoubleRow
  matmul (two kv tiles per instruction, 2x PE throughput; V stationary with
  a ones column folding the denominator, padded to stride 80 for the
  dual-fp8 ldweights 16B-alignment rule).  The diagonal pair's second tile
  is col-trimmed to [256, 512) (parity-uniform superset of the causal
  region).  AV lags the QK/exp stream by 2 pairs so PE never stalls on exp.
- ACT: exact exp for the diagonal pair (psum->bf16) and 3/5 of non-diag
  pairs (psum->fp8); table pre-warmed during input DMA; also issues the xq
  input DMAs (second HWDGE queue).
- DVE: exp for 2/5 of non-diag pairs via one tensor_scalar emitting fp8e4
  BITS as int8 (Schraudolph: bits = round(s*A8 + B8), ~+-6% per-element,
  max-norm safe since those pairs' outputs are softmax-diluted), q/k/v
  psum casts, av psum->sbuf output copies.
- Pool (gpsimd): diagonal mask multiplies, ones-column memsets (SBUF only;
  gpsimd cannot touch PSUM on TRN2).

Measured (8-core SPMD, max core): ~62-67us in the device's fast PE-clock
state, ~80-82us in its slow state (the tensor-engine clock is a per-run
lottery, ~385ns vs ~630ns per 512-col matmul; ACT is always stable).
Baseline was 81.9us.
"""

import sys

for _p in ("/opt/trn_rl_repo", "/root/.axon_site/_ro/trn_rl_repo"):
    if _p not in sys.path:
        sys.path.append(_p)

from contextlib import ExitStack

import numpy as np

import concourse.bacc as bacc
import concourse.bass as bass
import concourse.tile as tile
from concourse import mybir
from concourse.bass_utils import run_bass_kernel_spmd

B, T, C, H = 4, 4096, 256, 64
QB = 512         # query block width
NQB = T // QB    # 8 query blocks
KT = 128         # kv tile width
TK = T // 2      # gathered kv columns per core
F32 = mybir.dt.float32
BF16 = mybir.dt.bfloat16
FP8 = mybir.dt.float8e4
I16 = mybir.dt.int16
I8 = mybir.dt.int8
DR = mybir.MatmulPerfMode.DoubleRow
SCALE = float(C) ** -0.5
# Schraudolph: bits = round(s*A + B) ~ exp(s*SCALE), emitted as raw bf16/fp8 bits
SCH_A = 128.0 / float(np.log(2.0)) * SCALE
SCH_B = 16248.65
SCH_A8 = 8.0 / float(np.log(2.0)) * SCALE
SCH_B8 = 55.55
O1 = 256         # uniform col-trim offset for the diagonal pair's 2nd tile

_NC = None


def build_nc() -> bass.Bass:
    nc = bacc.Bacc("TRN2", target_bir_lowering=False, debug=False)
    xq = nc.declare_dram_parameter("xq", [128, 2, T], BF16, isOutput=False)
    xkv = nc.declare_dram_parameter("xkv", [128, 2, TK], BF16, isOutput=False)
    wq = nc.declare_dram_parameter("wq", [128, 2, 2 * H], BF16, isOutput=False)
    wkv = nc.declare_dram_parameter("wkv", [128, 2, 2 * H], BF16, isOutput=False)
    msk = nc.declare_dram_parameter("msk", [KT, QB], BF16, isOutput=False)
    ud = nc.declare_dram_parameter("ud", [H + 1, T], F32, isOutput=True)

    with tile.TileContext(nc) as tc, ExitStack() as ctx:
        persist = ctx.enter_context(tc.tile_pool(name="persist", bufs=1))
        pexp = ctx.enter_context(tc.tile_pool(name="exp", bufs=5))
        pout = ctx.enter_context(tc.tile_pool(name="out", bufs=2))
        pproj = ctx.enter_context(tc.tile_pool(name="pproj", bufs=2, space="PSUM"))
        pqk = ctx.enter_context(tc.tile_pool(name="pqk", bufs=2, space="PSUM"))
        pav = ctx.enter_context(tc.tile_pool(name="pav", bufs=2, space="PSUM"))

        # ---- input DMAs: need-first chunk order, split across SP+ACT queues --
        xkv_sb = persist.tile([128, 2, TK], BF16, tag="xkv")
        xq_sb = persist.tile([128, 2, T], BF16, tag="xq")
        # weights first (tiny, gate the first projections)
        w_sb = {}
        for name, dram in (("kv", wkv), ("q", wq)):
            t = persist.tile([128, 2, 2 * H], BF16, tag=f"w{name}")
            nc.sync.dma_start(out=t[:], in_=dram[:])
            w_sb[name] = t
        # first chunk split in four for parallel-queue latency
        for o in range(0, QB, 128):
            nc.sync.dma_start(
                out=xkv_sb[:, :, o : o + 128], in_=xkv[:, :, o : o + 128]
            )
        nc.scalar.dma_start(out=xq_sb[:, :, 0:256], in_=xq[:, :, 0:256])
        nc.scalar.dma_start(out=xq_sb[:, :, 256:QB], in_=xq[:, :, 256:QB])
        m_sb = persist.tile([KT, QB], BF16, tag="mask")
        nc.sync.dma_start(out=m_sb[:], in_=msk[:])
        # warm the ACT exp table while DMAs stream
        warm = persist.tile([1, 2], F32, tag="warm")
        nc.vector.memset(warm[:], 0.0)
        nc.scalar.activation(warm[:], warm[:], mybir.ActivationFunctionType.Exp)
        # remaining x chunks, interleaved need-first
        for j in range(1, NQB):
            nc.sync.dma_start(
                out=xq_sb[:, :, QB * j : QB * (j + 1)],
                in_=xq[:, :, QB * j : QB * (j + 1)],
            )
            if j < TK // QB:
                nc.sync.dma_start(
                    out=xkv_sb[:, :, QB * j : QB * (j + 1)],
                    in_=xkv[:, :, QB * j : QB * (j + 1)],
                )

        # ---- projections (bf16, contract C in 2 chunks) ---------------------
        q_sb = [None] * NQB           # bf16 [64, QB]
        k_sb = [None] * (TK // QB)    # bf16 [64, QB] local gathered layout
        v_sb = [None] * NQB           # bf16 [128, 2, 65] per pair (diag AV)
        v8_sb = [None] * NQB          # fp8 [128, 2, 80] per pair (DR AV)

        def proj_q(j):
            ps = pproj.tile([64, QB], F32, tag="proj")
            for c in range(2):
                nc.tensor.matmul(
                    ps[:], lhsT=w_sb["q"][:, c, 0:H],
                    rhs=xq_sb[:, c, QB * j : QB * (j + 1)],
                    start=(c == 0), stop=(c == 1),
                )
            t = persist.tile([64, QB], BF16, tag=f"q{j}")
            nc.vector.tensor_copy(t[:], ps[:])
            q_sb[j] = t

        def proj_kv(j):
            ps = pproj.tile([64, QB], F32, tag="proj")
            for c in range(2):
                nc.tensor.matmul(
                    ps[:], lhsT=w_sb["kv"][:, c, 0:H],
                    rhs=xkv_sb[:, c, QB * j : QB * (j + 1)],
                    start=(c == 0), stop=(c == 1),
                )
            t = persist.tile([64, QB], BF16, tag=f"kv{j}")
            nc.vector.tensor_copy(t[:], ps[:])
            k_sb[j] = t

        def proj_v(P):
            ps = pproj.tile([128, 2, H], F32, tag="proj")
            for h in range(2):
                s = 2 * P + h
                for c in range(2):
                    nc.tensor.matmul(
                        ps[:, h, :],
                        lhsT=xkv_sb[:, c, KT * s : KT * (s + 1)],
                        rhs=w_sb["kv"][:, c, H : 2 * H],
                        start=(c == 0), stop=(c == 1),
                    )
            t = persist.tile([128, 2, H + 1], BF16, tag=f"v{P}")
            nc.vector.tensor_copy(t[:, :, 0:H], ps[:])
            nc.gpsimd.memset(t[:, :, H : H + 1], 1.0)
            v_sb[P] = t
            t8 = persist.tile([128, 2, H + 16], FP8, tag=f"v8{P}")
            nc.vector.tensor_copy(t8[:, :, 0:H], ps[:])
            nc.gpsimd.memset(t8[:, :, H : H + 16], 1.0)
            v8_sb[P] = t8

        for j in range(TK // QB):
            proj_kv(j)
            proj_v(2 * j)
            proj_v(2 * j + 1)
            proj_q(2 * j)
            proj_q(2 * j + 1)

        # ---- attention -------------------------------------------------------
        def k_slice(s):  # local kv tile s -> gathered k columns
            return k_sb[s // 4][:, KT * (s % 4) : KT * (s % 4 + 1)]

        nslot = [0]

        def emit_qk_exp(p, P):
            diag = P == p
            if not diag:
                nslot[0] += 1
            qk2 = pqk.tile([KT, 2 * QB], F32, tag="qk")
            nc.tensor.matmul(
                qk2[:, 0:QB], lhsT=k_slice(2 * P), rhs=q_sb[p][:],
                start=True, stop=True,
            )
            if diag:
                nc.tensor.matmul(
                    qk2[:, QB + O1 : 2 * QB], lhsT=k_slice(2 * P + 1),
                    rhs=q_sb[p][:, O1:QB], start=True, stop=True,
                )
            else:
                nc.tensor.matmul(
                    qk2[:, QB : 2 * QB], lhsT=k_slice(2 * P + 1), rhs=q_sb[p][:],
                    start=True, stop=True,
                )
            if diag:
                ex = pexp.tile([KT, 2 * QB], BF16, tag="exp")
                nc.scalar.activation(
                    ex[:, 0:QB], qk2[:, 0:QB],
                    mybir.ActivationFunctionType.Exp, scale=SCALE,
                )
                nc.scalar.activation(
                    ex[:, QB + O1 : 2 * QB], qk2[:, QB + O1 : 2 * QB],
                    mybir.ActivationFunctionType.Exp, scale=SCALE,
                )
                # masks: region h0 = ex[:, 0:256] (*= msk[:, 0:256]),
                #        region h1 = ex[:, 768:1024] (*= msk[:, 256:512])
                nc.gpsimd.tensor_mul(ex[:, 0:O1], ex[:, 0:O1], m_sb[:, 0:O1])
                nc.gpsimd.tensor_mul(
                    ex[:, QB + O1 : 2 * QB], ex[:, QB + O1 : 2 * QB],
                    m_sb[:, O1:QB],
                )
                return ex
            ex = pexp.tile([KT, 2 * QB], FP8, tag="exp8")
            if nslot[0] % 5 < 3:
                nc.scalar.activation(
                    ex[:], qk2[:], mybir.ActivationFunctionType.Exp, scale=SCALE
                )
            else:
                nc.vector.tensor_scalar(
                    ex[:].bitcast(I8), qk2[:], SCH_A8, SCH_B8,
                    mybir.AluOpType.mult, mybir.AluOpType.add,
                )
            return ex

        av_tiles = {}

        def emit_av(p, P, ex):
            diag = P == p
            av = av_tiles[p]
            if diag:
                nc.tensor.matmul(
                    av[:], lhsT=v_sb[P][:, 0, :], rhs=ex[:, 0:QB],
                    start=(P == 0), stop=False,
                )
                nc.tensor.matmul(
                    av[:, O1:QB], lhsT=v_sb[P][:, 1, :],
                    rhs=ex[:, QB + O1 : 2 * QB], start=False, stop=True,
                )
            else:
                nc.tensor.matmul(
                    av_pad_tiles[p][:], lhsT=v8_sb[P][:, :, :],
                    rhs=ex[:].rearrange("p (two n) -> p two n", two=2),
                    start=(P == 0), stop=False, perf_mode=DR,
                )
            if diag:  # block finished: drain, DMA out
                ot = pout.tile([H + 1, QB], F32, tag="out")
                if p == NQB - 1:
                    # last block: split the drain so copy/DMA/transfer pipeline
                    for o in range(0, QB, 128):
                        nc.vector.tensor_copy(ot[:, o : o + 128], av[:, o : o + 128])
                        nc.sync.dma_start(
                            out=ud[:, QB * p + o : QB * p + o + 128],
                            in_=ot[:, o : o + 128],
                        )
                else:
                    nc.vector.tensor_copy(ot[:], av[:])
                    nc.sync.dma_start(
                        out=ud[:, QB * p : QB * (p + 1)], in_=ot[:]
                    )

        av_pad_tiles = {}
        pending = []
        for p in range(NQB):
            av = pav.tile([H + 16, QB], F32, tag="av")
            av_pad_tiles[p] = av
            av_tiles[p] = av[0 : H + 1, :]
            for P in range(p + 1):
                ex = emit_qk_exp(p, P)
                pending.append((p, P, ex))
                if len(pending) > 2:
                    emit_av(*pending.pop(0))
        while pending:
            emit_av(*pending.pop(0))

    nc.compile()
    return nc


def get_nc() -> bass.Bass:
    global _NC
    if _NC is None:
        _NC = build_nc()
    return _NC


def make_in_maps(x, Wk, Wq, Wv):
    import ml_dtypes

    bf16 = ml_dtypes.bfloat16
    x = np.asarray(x, np.float32)

    def wpack(Wl, Wr):
        Wm = np.concatenate(
            [np.asarray(Wl, np.float32), np.asarray(Wr, np.float32)], axis=1
        )
        return np.ascontiguousarray(
            Wm.reshape(2, 128, 2 * H).transpose(1, 0, 2)
        ).astype(bf16)

    wq8 = wpack(Wq, np.zeros_like(Wq))
    wkv8 = wpack(Wk, Wv)

    kk = np.arange(KT)[:, None]
    jj = np.arange(QB)[None, :]
    in_maps = []
    for core in range(8):
        b, par = divmod(core, 2)
        xb = x[b].T.reshape(2, 128, T).transpose(1, 0, 2)  # [128, 2, T]
        xq = np.ascontiguousarray(xb).astype(bf16)
        # gathered parity columns: local tile s -> global tile g=2s+par
        cols = (
            (2 * np.arange(TK // KT)[:, None] + par) * KT + np.arange(KT)[None, :]
        ).reshape(-1)
        xkv = np.ascontiguousarray(xb[:, :, cols]).astype(bf16)
        # mask [128, 512]: cols 0:256 for diag tile d0 (offset 128*par),
        # cols 256:512 for diag tile d1 (offset 256+128*par), both relative
        # to the computed regions (h0 cols 0:256 of q-block, h1 cols 256:512).
        m = np.zeros((KT, QB), np.float32)
        m[:, 0:O1] = (jj[:, 0:O1] >= kk + 128 * par).astype(np.float32)
        m[:, O1:QB] = (jj[:, O1:QB] >= kk + O1 + 128 * par).astype(np.float32)
        in_maps.append(
            {"xq": xq, "xkv": xkv, "wq": wq8, "wkv": wkv8,
             "msk": m.astype(bf16)}
        )
    return in_maps


def merge(results):
    out = np.empty((B, T, H), np.float32)
    for b in range(B):
        s = results[2 * b]["ud"] + results[2 * b + 1]["ud"]  # [65, T]
        out[b] = (s[0:H] / s[H : H + 1]).T
    return out


def kernel(x, Wk, Wq, Wv, **kw):
    in_maps = make_in_maps(x, Wk, Wq, Wv)
    res = run_bass_kernel_spmd(get_nc(), in_maps, core_ids=list(range(8)), **kw)
    out = merge(res.results)
    if kw:
        return out, res
    return out
